# revision 1
# baseline (speedup 1.0000x reference)
"""Mamba (selective SSM) layer on 8 Trainium2 NeuronCores via Bass/Tile.

Sharding: tensor-parallel over d_inner (512 channels/core). x_proj partial
sums AllReduced (2050x96) so every core sees full dt_in/B/C. Scan layout:
[d on partitions, time on free], DVE tensor_tensor_scan per (d-tile, n),
A folded into ACT exp scale (A rows constant across d for S4D init; general
fallback uses per-partition scale vectors). Host sums out_proj partials.
"""
import numpy as np
import ml_dtypes

import concourse.bass as bass
import concourse.bacc as bacc
import concourse.tile as tile
from concourse import mybir
from concourse.bass_utils import run_bass_kernel_spmd

F32 = mybir.dt.float32
F32R = mybir.dt.float32r
BF16 = mybir.dt.bfloat16
AF = mybir.ActivationFunctionType
OP = mybir.AluOpType

B = 2
L = 1025            # seq len incl. prepended emb token
TT = B * L          # 2050 tokens, b-major
DM = 1024
DI = 4096
N_CORES = 8
DLOC = DI // N_CORES  # 512
NDT = DLOC // 128     # 4 d-tiles
NST = 16
DTR = 64
XD = DTR + 2 * NST    # 96

TCH = [512, 512, 512, 512, 2]
TCHO = [0, 512, 1024, 1536, 2048]


def _tslices():
    out, t = [], 0
    while t < TT:
        w = min(128, TT - t)
        out.append((t, w))
        t += w
    return out


def build_program(a_scales, debug=False, sim_no_cc=False):
    nc = bacc.Bacc(trn_type="TRN2")

    xsT = nc.dram_tensor("xsT", [DM, TT], F32R, kind="ExternalInput")
    w_in = nc.dram_tensor("w_in", [DM, 2 * DLOC], F32R, kind="ExternalInput")
    conv_w = nc.dram_tensor("conv_w", [DLOC, 4], F32, kind="ExternalInput")
    conv_b = nc.dram_tensor("conv_b", [DLOC, 1], F32, kind="ExternalInput")
    w_x = nc.dram_tensor("w_x", [DLOC, XD], BF16, kind="ExternalInput")
    w_dt = nc.dram_tensor("w_dt", [DTR, DLOC], BF16, kind="ExternalInput")
    b_dt = nc.dram_tensor("b_dt", [DLOC, 1], F32, kind="ExternalInput")
    a_cols = nc.dram_tensor("a_cols", [DLOC, NST], F32, kind="ExternalInput")
    d_skip = nc.dram_tensor("d_skip", [DLOC, 1], F32, kind="ExternalInput")
    w_out = nc.dram_tensor("w_out", [DLOC, DM], BF16, kind="ExternalInput")
    ident = nc.dram_tensor("ident", [128, 128], F32, kind="ExternalInput")
    out_p = nc.dram_tensor("out_p", [TT, DM], F32, kind="ExternalOutput")

    if debug:
        dbg = {nm: nc.dram_tensor(f"dbg_{nm}", [128, TT], F32, kind="ExternalOutput")
               for nm in ("xi", "xh", "dt", "y", "da", "u", "h")}
        dbg["xdbl"] = nc.dram_tensor("dbg_xdbl", [TT, XD], F32, kind="ExternalOutput")
    with tile.TileContext(nc) as tc:
        with (
            tc.tile_pool(name="wts", bufs=1) as wts,
            tc.tile_pool(name="mem", bufs=1) as mem,
            tc.tile_pool(name="ps", bufs=1, space="PSUM") as ps,
            tc.tile_pool(name="dram", bufs=1, space="DRAM") as dram,
        ):
            ar_in = dram.tile([TT, XD], F32, name="ar_in")
            ar_out = dram.tile([TT, XD], F32, name="ar_out", addr_space="Shared")
            bc_rows = dram.tile([2 * NST, TT], BF16, name="bc_rows")
            # ---------- small persistent weights ----------
            sb_ident = wts.tile([128, 128], F32)
            nc.sync.dma_start(out=sb_ident, in_=ident[:, :])
            sb_cw, sb_cb, sb_bdt, sb_dsk, sb_wx, sb_acol = [], [], [], [], [], []
            for d in range(NDT):
                sl = slice(d * 128, (d + 1) * 128)
                for lst, src, w in ((sb_cw, conv_w, 4), (sb_cb, conv_b, 1),
                                    (sb_bdt, b_dt, 1), (sb_dsk, d_skip, 1)):
                    t = wts.tile([128, w], F32, name=f"w{len(lst)}_{id(src) % 997}_{d}")
                    nc.sync.dma_start(out=t, in_=src[sl, :])
                    lst.append(t)
                t = wts.tile([128, XD], BF16, name=f"wx{d}")
                nc.sync.dma_start(out=t, in_=w_x[sl, :])
                sb_wx.append(t)
                t = wts.tile([128, NST], F32, name=f"ac{d}")
                nc.sync.dma_start(out=t, in_=a_cols[sl, :])
                sb_acol.append(t)
            sb_wdt = wts.tile([DTR, DLOC], BF16)
            nc.sync.dma_start(out=sb_wdt, in_=w_dt[:, :])

            # ---------- persistent activations (bf16) ----------
            sb_xh = [mem.tile([128, TT], BF16, name=f"xh{d}", tag=f"xh{d}")
                     for d in range(NDT)]
            sb_z = [mem.tile([128, TT], BF16, name=f"z{d}", tag=f"z{d}")
                    for d in range(NDT)]
            sb_dtx = [mem.tile([128, TT], BF16, name=f"dtx{d}", tag=f"dtx{d}")
                      for d in range(NDT)]
            sb_y = [mem.tile([128, TT], BF16, name=f"y{d}", tag=f"y{d}")
                    for d in range(NDT)]

            # ---------- Phase 1: in_proj (f32r) -> xi (slab), z ----------
            sb_xi = [mem.tile([128, TT], F32, name=f"xi{d}", tag="slab", bufs=7)
                     for d in range(NDT)]
            for ci, cw in enumerate(TCH):
                co = TCHO[ci]
                pts = [ps.tile([128, 512], F32, name=f"pj{ci}_{m}", tag="pj", bufs=8)
                       for m in range(8)]
                for k in range(8):
                    xsc = mem.tile([128, 512], F32R, name=f"xsc{ci}_{k}",
                                   tag="xsc", bufs=3)
                    nc.sync.dma_start(out=xsc[:, :cw],
                                      in_=xsT[k * 128:(k + 1) * 128, co:co + cw])
                    wic = mem.tile([128, 1024], F32R, name=f"wic{ci}_{k}",
                                   tag="wic", bufs=3)
                    nc.sync.dma_start(out=wic, in_=w_in[k * 128:(k + 1) * 128, :])
                    for m in range(8):
                        nc.tensor.matmul(pts[m][:, :cw],
                                         wic[:, m * 128:(m + 1) * 128],
                                         xsc[:, :cw],
                                         start=(k == 0), stop=(k == 7))
                for m in range(8):
                    if m < 4:
                        nc.scalar.copy(sb_xi[m][:, co:co + cw], pts[m][:, :cw])
                    else:
                        nc.scalar.activation(sb_z[m - 4][:, co:co + cw],
                                             pts[m][:, :cw], AF.Silu)

            # ---------- Phase 2: causal depthwise conv + silu -> xh ----------
            for d in range(NDT):
                xc = mem.tile([128, TT], F32, name=f"xc{d}", tag="slab", bufs=7)
                nc.vector.tensor_scalar_mul(xc, sb_xi[d], sb_cw[d][:, 3:4])
                for b in range(B):
                    s = b * L
                    for j in range(3):
                        o = 3 - j
                        tp = mem.tile([128, L], F32, name=f"tp{d}_{b}_{j}",
                                      tag="tap", bufs=2)
                        nc.scalar.activation(tp[:, :L - o], sb_xi[d][:, s:s + L - o],
                                             AF.Copy, scale=sb_cw[d][:, j:j + 1])
                        nc.vector.tensor_tensor(xc[:, s + o:s + L],
                                                xc[:, s + o:s + L],
                                                tp[:, :L - o], OP.add)
                nc.scalar.activation(sb_xh[d], xc, AF.Silu, bias=sb_cb[d])

            if debug:
                nc.sync.dma_start(out=dbg["xi"][:, :], in_=sb_xi[0])
                dxh = mem.tile([128, TT], F32, name="dxh", tag="slab", bufs=7)
                nc.vector.tensor_copy(dxh, sb_xh[0])
                nc.sync.dma_start(out=dbg["xh"][:, :], in_=dxh)
            # ---------- Phase 3: x_proj partials + AllReduce ----------
            for ti, (t0, twd) in enumerate(_tslices()):
                pt = ps.tile([128, XD], F32, name=f"px{ti}", tag="pj", bufs=8)
                for d in range(NDT):
                    nc.tensor.matmul(pt[:twd, :], sb_xh[d][:, t0:t0 + twd],
                                     sb_wx[d], start=(d == 0), stop=(d == NDT - 1))
                ev = mem.tile([128, XD], F32, name=f"xde{ti}", tag="xde", bufs=3)
                nc.scalar.copy(ev[:twd, :], pt[:twd, :])
                nc.sync.dma_start(out=ar_in[t0:t0 + twd, :], in_=ev[:twd, :])
            if sim_no_cc:
                nc.sync.dma_start(out=ar_out[:, :], in_=ar_in[:, :])
            else:
                nc.gpsimd.collective_compute(
                    "AllReduce", OP.add, replica_groups=[list(range(N_CORES))],
                    ins=[ar_in.opt()], outs=[ar_out.opt()])

            # ---------- Phase 4: transpose x_dbl -> [96, t] bf16 ----------
            sb_xdT = mem.tile([96, TT], BF16, tag="bb", bufs=2)
            for ti, (t0, twd) in enumerate(_tslices()):
                ld = mem.tile([128, XD], F32, name=f"xl{ti}", tag="xde", bufs=3)
                nc.sync.dma_start(out=ld[:twd, :], in_=ar_out[t0:t0 + twd, :])
                pt = ps.tile([128, 128], F32, name=f"ptr{ti}", tag="pj", bufs=8)
                nc.tensor.transpose(pt[:XD, :twd], ld[:twd, :XD],
                                    sb_ident[:twd, :twd])
                nc.scalar.copy(sb_xdT[:, t0:t0 + twd], pt[:XD, :twd])
            nc.sync.dma_start(out=bc_rows[:, :], in_=sb_xdT[DTR:XD, :])

            if debug:
                nc.sync.dma_start(out=dbg["xdbl"][:, :], in_=ar_out[:, :])
            # ---------- Phase 5: dt (softplus) f32, dtx bf16 ----------
            sb_dt = [mem.tile([128, TT], F32, name=f"dt{d}", tag="slab", bufs=7)
                     for d in range(NDT)]
            for d in range(NDT):
                for ci, cw in enumerate(TCH):
                    co = TCHO[ci]
                    pt = ps.tile([128, 512], F32, name=f"pd{d}_{ci}", tag="pj",
                                 bufs=8)
                    nc.tensor.matmul(pt[:, :cw], sb_wdt[:, d * 128:(d + 1) * 128],
                                     sb_xdT[:DTR, co:co + cw], start=True, stop=True)
                    e1 = mem.tile([128, 512], F32, name=f"e{d}_{ci}", tag="sp",
                                  bufs=3)
                    nc.scalar.activation(e1[:, :cw], pt[:, :cw], AF.Exp,
                                         bias=sb_bdt[d])
                    nc.scalar.activation(sb_dt[d][:, co:co + cw], e1[:, :cw],
                                         AF.Ln, bias=1.0)
                nc.vector.tensor_tensor(sb_dtx[d], sb_dt[d], sb_xh[d], OP.mult)

            if debug:
                nc.sync.dma_start(out=dbg["dt"][:, :], in_=sb_dt[0])
            # ---------- Phase 6: selective scan ----------
            for n in range(NST):
                bb = mem.tile([128, TT], BF16, name=f"bb{n}", tag="bb", bufs=2)
                cb = mem.tile([128, TT], BF16, name=f"cb{n}", tag="cbx", bufs=2)
                for dst, row in ((bb, n), (cb, NST + n)):
                    src = bc_rows[row:row + 1, :]
                    nc.sync.dma_start(out=dst, in_=bass.AP(
                        tensor=src.tensor, offset=src.offset,
                        ap=[[0, 128]] + src.ap[1:]))
                for d in range(NDT):
                    da = mem.tile([128, TT], F32, name=f"da{n}_{d}", tag="slab",
                                  bufs=7)
                    if a_scales is not None:
                        nc.scalar.activation(da, sb_dt[d], AF.Exp,
                                             scale=float(a_scales[n]))
                    else:
                        nc.scalar.activation(da, sb_dt[d], AF.Exp,
                                             scale=sb_acol[d][:, n:n + 1])
                    nc.vector.memset(da[:, L:L + 1], 0.0)
                    u = mem.tile([128, TT], BF16, name=f"u{n}_{d}", tag="u", bufs=2)
                    nc.vector.tensor_tensor(u, sb_dtx[d], bb, OP.mult)
                    h = mem.tile([128, TT], BF16, name=f"h{n}_{d}", tag="h", bufs=2)
                    nc.vector.tensor_tensor_scan(h, da, u, 0.0, OP.mult, OP.add)
                    p = mem.tile([128, TT], BF16, name=f"p{n}_{d}", tag="p", bufs=2)
                    nc.vector.tensor_tensor(p, h, cb, OP.mult)
                    if debug and n == 0 and d == 0:
                        ddump = mem.tile([128, TT], F32, name="ddmp", tag="slab", bufs=7)
                        nc.vector.tensor_copy(ddump, da)
                        nc.sync.dma_start(out=dbg["da"][:, :], in_=ddump)
                        udump = mem.tile([128, TT], F32, name="udmp", tag="slab", bufs=7)
                        nc.vector.tensor_copy(udump, u)
                        nc.sync.dma_start(out=dbg["u"][:, :], in_=udump)
                        hdump = mem.tile([128, TT], F32, name="hdmp", tag="slab", bufs=7)
                        nc.vector.tensor_copy(hdump, h)
                        nc.sync.dma_start(out=dbg["h"][:, :], in_=hdump)
                    if n == 0:
                        nc.vector.tensor_copy(sb_y[d], p)
                    else:
                        nc.vector.tensor_tensor(sb_y[d], sb_y[d], p, OP.add)

            if debug:
                dy = mem.tile([128, TT], F32, name="dy", tag="slab", bufs=7)
                nc.vector.tensor_copy(dy, sb_y[0])
                nc.sync.dma_start(out=dbg["y"][:, :], in_=dy)
            # ---------- Phase 7: skip + gate (into z slot) ----------
            for d in range(NDT):
                sk = mem.tile([128, TT], BF16, name=f"sk{d}", tag="u", bufs=2)
                nc.vector.tensor_scalar_mul(sk, sb_xh[d], sb_dsk[d][:, 0:1])
                nc.vector.tensor_tensor(sk, sb_y[d], sk, OP.add)
                nc.vector.tensor_tensor(sb_z[d], sk, sb_z[d], OP.mult)

            # ---------- Phase 8: out_proj partials (bf16) ----------
            sb_wo = []
            for d in range(NDT):
                t = wts.tile([128, DM], BF16, name=f"wo{d}")
                nc.sync.dma_start(out=t, in_=w_out[d * 128:(d + 1) * 128, :])
                sb_wo.append(t)
            for ti, (t0, twd) in enumerate(_tslices()):
                for f in range(2):
                    pt = ps.tile([128, 512], F32, name=f"po{ti}_{f}", tag="pj",
                                 bufs=8)
                    for d in range(NDT):
                        nc.tensor.matmul(
                            pt[:twd, :], sb_z[d][:, t0:t0 + twd],
                            sb_wo[d][:, f * 512:(f + 1) * 512],
                            start=(d == 0), stop=(d == NDT - 1))
                    ev = mem.tile([128, 512], F32, name=f"oe{ti}_{f}", tag="sp",
                                  bufs=3)
                    nc.scalar.copy(ev[:twd, :], pt[:twd, :])
                    nc.sync.dma_start(out=out_p[t0:t0 + twd, f * 512:(f + 1) * 512],
                                      in_=ev[:twd, :])

    nc.compile()
    return nc


_CACHE = {}


def _get_program(a_scales_key):
    if a_scales_key not in _CACHE:
        _CACHE[a_scales_key] = build_program(
            list(a_scales_key) if a_scales_key is not None else None)
    return _CACHE[a_scales_key]


def make_inputs(x, layer_idx, emb_table, W_in, conv_w, conv_b, W_x, W_dt, b_dt,
                A_log, D_skip, W_out):
    x = np.asarray(x, np.float32)
    emb = np.asarray(emb_table, np.float32)[int(layer_idx)]
    xs = np.concatenate([np.broadcast_to(emb, (B, 1, DM)), x], axis=1)
    xsT = np.ascontiguousarray(xs.reshape(TT, DM).T)

    A = -np.exp(np.asarray(A_log, np.float64)).astype(np.float32)
    same = bool(np.all(A == A[0:1, :]))
    a_key = tuple(float(v) for v in A[0]) if same else None

    W_in = np.asarray(W_in, np.float32)
    ident = np.eye(128, dtype=np.float32)
    ins = []
    for c in range(N_CORES):
        sl = slice(c * DLOC, (c + 1) * DLOC)
        w_in_cat = np.concatenate(
            [W_in[:, c * DLOC:(c + 1) * DLOC],
             W_in[:, DI + c * DLOC:DI + (c + 1) * DLOC]], axis=1)
        ins.append({
            "xsT": xsT,
            "w_in": np.ascontiguousarray(w_in_cat),
            "conv_w": np.ascontiguousarray(np.asarray(conv_w, np.float32)[sl]),
            "conv_b": np.ascontiguousarray(
                np.asarray(conv_b, np.float32)[sl][:, None]),
            "w_x": np.ascontiguousarray(np.asarray(W_x, np.float32)[sl]).astype(ml_dtypes.bfloat16),
            "w_dt": np.ascontiguousarray(np.asarray(W_dt, np.float32)[:, sl]).astype(ml_dtypes.bfloat16),
            "b_dt": np.ascontiguousarray(
                np.asarray(b_dt, np.float32)[sl][:, None]),
            "a_cols": np.ascontiguousarray(A[sl]),
            "d_skip": np.ascontiguousarray(
                np.asarray(D_skip, np.float32)[sl][:, None]),
            "w_out": np.ascontiguousarray(np.asarray(W_out, np.float32)[sl]).astype(ml_dtypes.bfloat16),
            "ident": ident,
        })
    return ins, a_key


def kernel(**inputs) -> np.ndarray:
    ins, a_key = make_inputs(**inputs)
    nc = _get_program(a_key)
    res = run_bass_kernel_spmd(nc, ins, core_ids=list(range(N_CORES)))
    out = np.zeros((TT, DM), np.float64)
    for c in range(N_CORES):
        out += res.results[c]["out_p"]
    return out.astype(np.float32).reshape(B, L, DM)



# revision 36
# speedup vs baseline: 7.6483x; 7.6483x over previous
"""Mamba (selective SSM) layer on 8 Trainium2 NeuronCores via Bass/Tile.

v4: tensor-parallel over d_inner (512 ch/core), all-bf16, wide-tile layout:
the 4 d-tiles of a batch live side-by-side in the free dim ([128, 4*1025]),
so each scan-block step (u-mult, scan, p-mult, y-accum) is ONE wide DVE op
per (batch, state) instead of 4 - 4x fewer ops and semaphores. The wide
scan stays correct across segment boundaries because the decay da is
zeroed at each segment's first column (state reset). B/C rows feed the
wide ops via stride-0 free-replication APs (no widened broadcast).
Per-batch pipeline: front(in_proj,conv,x_proj)+AllReduce per batch;
batch-1's collective and dt-phase overlap batch-0's scan block; out_proj
of batch 0 overlaps batch-1's block. Host sums f32 out_p partials.
reps>1 unrolls the body in-NEFF for slope-based device timing.
"""
import numpy as np
import ml_dtypes

import concourse.bass as bass
import concourse.bacc as bacc
import concourse.tile as tile
from concourse import mybir
from concourse.bass_utils import run_bass_kernel_spmd

F32 = mybir.dt.float32
BF16 = mybir.dt.bfloat16
AF = mybir.ActivationFunctionType
OP = mybir.AluOpType

B = 2
L = 1025            # tokens per batch incl. prepended emb token
TT = B * L          # 2050
DM = 1024
DI = 4096
N_CORES = 8
DLOC = DI // N_CORES  # 512
NDT = DLOC // 128     # 4 d-segments
WL = NDT * L          # 4100 wide free dim
NST = 16
DTR = 64
XD = DTR + 2 * NST    # 96

BCH = [(0, 512), (512, 512), (1024, 1)]   # per-batch token chunks


def _bslices():
    out, t = [], 0
    while t < L:
        w = min(128, L - t)
        out.append((t, w))
        t += w
    return out


def _rep(t, nseg):
    """Present [128, w] tile as [128, nseg*w] via stride-0 segment dim."""
    a = t[:, :]
    return bass.AP(tensor=a.tensor, offset=a.offset,
                   ap=[a.ap[0], [0, nseg], a.ap[1]])


def _segs(t, o, w):
    """[128, NDT segments of width w starting at offset o within each L]."""
    a = t[:, o:]
    return bass.AP(tensor=a.tensor, offset=a.offset,
                   ap=[a.ap[0], [L, NDT], [1, w]])


def build_program(a_scales, debug=False, sim_no_cc=False, cc_mode="cc", reps=1,
                  ablate=()):
    if sim_no_cc:
        cc_mode = "copy"
    ablate = set(ablate)
    nc = bacc.Bacc(trn_type="TRN2")

    xsT = nc.dram_tensor("xsT", [DM, TT], BF16, kind="ExternalInput")
    w_in = nc.dram_tensor("w_in", [DM, 2 * DLOC], BF16, kind="ExternalInput")
    conv_w = nc.dram_tensor("conv_w", [DLOC, 4], F32, kind="ExternalInput")
    conv_b = nc.dram_tensor("conv_b", [DLOC, 1], F32, kind="ExternalInput")
    w_x = nc.dram_tensor("w_x", [DLOC, XD], BF16, kind="ExternalInput")
    w_dt = nc.dram_tensor("w_dt", [DTR, DLOC], BF16, kind="ExternalInput")
    b_dt = nc.dram_tensor("b_dt", [DLOC, 1], F32, kind="ExternalInput")
    a_cols = nc.dram_tensor("a_cols", [DLOC, NST], F32, kind="ExternalInput")
    d_skip = nc.dram_tensor("d_skip", [DLOC, 1], F32, kind="ExternalInput")
    w_out = nc.dram_tensor("w_out", [DLOC, DM], BF16, kind="ExternalInput")
    identb = nc.dram_tensor("identb", [128, 128], BF16, kind="ExternalInput")
    out_p = nc.dram_tensor("out_p", [TT, DM], F32, kind="ExternalOutput")

    with tile.TileContext(nc) as tc:
        with (
            tc.tile_pool(name="wts", bufs=1) as wts,
            tc.tile_pool(name="mem", bufs=1) as mem,
            tc.tile_pool(name="ps", bufs=1, space="PSUM") as ps,
            tc.tile_pool(name="dram", bufs=1, space="DRAM") as dram,
        ):
            sb_ident = wts.tile([128, 128], BF16)
            nc.sync.dma_start(out=sb_ident, in_=identb[:, :])
            sb_cw, sb_cb, sb_bdt, sb_dsk, sb_wx, sb_acol = [], [], [], [], [], []
            for d in range(NDT):
                sl = slice(d * 128, (d + 1) * 128)
                for lst, src, w in ((sb_cw, conv_w, 4), (sb_cb, conv_b, 1),
                                    (sb_bdt, b_dt, 1), (sb_dsk, d_skip, 1)):
                    t = wts.tile([128, w], F32, name=f"w{len(lst)}_{id(src) % 997}_{d}")
                    nc.sync.dma_start(out=t, in_=src[sl, :])
                    lst.append(t)
                t = wts.tile([128, XD], BF16, name=f"wx{d}")
                nc.sync.dma_start(out=t, in_=w_x[sl, :])
                sb_wx.append(t)
                t = wts.tile([128, NST], F32, name=f"ac{d}")
                nc.sync.dma_start(out=t, in_=a_cols[sl, :])
                sb_acol.append(t)
            sb_wdt = wts.tile([DTR, DLOC], BF16)
            nc.sync.dma_start(out=sb_wdt, in_=w_dt[:, :])
            sb_wo = []
            for d in range(NDT):
                t = wts.tile([128, DM], BF16, name=f"wo{d}")
                nc.sync.dma_start(out=t, in_=w_out[d * 128:(d + 1) * 128, :])
                sb_wo.append(t)

            W = dict(ident=sb_ident, cw=sb_cw, cb=sb_cb, bdt=sb_bdt,
                     dsk=sb_dsk, wx=sb_wx, acol=sb_acol, wdt=sb_wdt, wo=sb_wo)
            for rep in range(reps):
                ar_in = [dram.tile([L, XD], BF16, name=f"ar_in{rep}_{b}")
                         for b in range(B)]
                ar_out = [dram.tile([L, XD], BF16, name=f"ar_out{rep}_{b}",
                                    addr_space="Shared") for b in range(B)]
                bc_rows = [dram.tile([2 * NST, L], BF16, name=f"bc{rep}_{b}")
                           for b in range(B)]
                body(nc, rep, W, xsT, w_in, out_p, ar_in, ar_out, bc_rows,
                     mem, ps, a_scales, cc_mode, ablate)

    nc.compile()
    return nc


def body(nc, rep, W, xsT, w_in, out_p, ar_in, ar_out, bc_rows, mem, ps,
         a_scales, cc_mode, ablate=frozenset()):
    R = f"r{rep}_"
    # wide persistents per batch; xi shares buffers with y (xi dies at conv,
    # y is born in the scan block)
    sb_xi = [mem.tile([128, WL], BF16, name=f"{R}xi{b}", tag="xiy", bufs=2)
             for b in range(B)]
    sb_xh = [mem.tile([128, WL], BF16, name=f"{R}xh{b}", tag="xh", bufs=2)
             for b in range(B)]
    sb_z = [mem.tile([128, WL], BF16, name=f"{R}z{b}", tag="z", bufs=2)
            for b in range(B)]
    sb_dt = [mem.tile([128, WL], BF16, name=f"{R}dt{b}", tag="dtw", bufs=2)
             for b in range(B)]
    sb_dtx = [mem.tile([128, WL], BF16, name=f"{R}dtx{b}", tag="dtx", bufs=2)
              for b in range(B)]
    sb_y = None  # allocated per batch in block() from the xiy ring

    if "noproj1" in ablate:
        for t in sb_xi + sb_z:
            nc.vector.memset(t, 0.0)
    if "nodt" in ablate:
        for t in sb_dt + sb_dtx:
            nc.vector.memset(t, 0.0)

    def front(b):
        s = b * L
        for ci, (co, cw) in enumerate(BCH) if "noproj1" not in ablate else ():
            pts = [ps.tile([128, 512], F32, name=f"{R}pj{b}_{ci}_{m}",
                           tag="pj", bufs=8) for m in range(8)]
            for k in range(8):
                xsc = mem.tile([128, 512], BF16, name=f"{R}xsc{b}_{ci}_{k}",
                               tag="xsc", bufs=4)
                nc.sync.dma_start(out=xsc[:, :cw],
                                  in_=xsT[k * 128:(k + 1) * 128,
                                          s + co:s + co + cw])
                wic = mem.tile([128, 1024], BF16, name=f"{R}wic{b}_{ci}_{k}",
                               tag="wic", bufs=3)
                nc.sync.dma_start(out=wic, in_=w_in[k * 128:(k + 1) * 128, :])
                for m in range(8):
                    nc.tensor.matmul(pts[m][:, :cw],
                                     wic[:, m * 128:(m + 1) * 128],
                                     xsc[:, :cw],
                                     start=(k == 0), stop=(k == 7))
            for m in range(8):
                if m < 4:
                    nc.scalar.copy(sb_xi[b][:, m * L + co:m * L + co + cw],
                                   pts[m][:, :cw])
                else:
                    nc.scalar.activation(
                        sb_z[b][:, (m - 4) * L + co:(m - 4) * L + co + cw],
                        pts[m][:, :cw], AF.Silu)

        # causal depthwise conv + silu -> xh (per-segment scalars, wide adds)
        if "noconv" in ablate:
            for d in range(NDT):
                nc.scalar.activation(sb_xh[b][:, d * L:(d + 1) * L],
                                     sb_xi[b][:, d * L:(d + 1) * L],
                                     AF.Silu, bias=W["cb"][d])
        else:
            xc = mem.tile([128, WL], BF16, name=f"{R}xc{b}", tag="h", bufs=2)
            for d in range(NDT):
                nc.vector.tensor_scalar_mul(xc[:, d * L:(d + 1) * L],
                                            sb_xi[b][:, d * L:(d + 1) * L],
                                            W["cw"][d][:, 3:4])
            for j in range(3):
                o = 3 - j
                tp = mem.tile([128, WL], BF16, name=f"{R}tp{b}_{j}", tag="u",
                              bufs=3)
                for d in range(NDT):
                    nc.scalar.activation(tp[:, d * L:d * L + L - o],
                                         sb_xi[b][:, d * L:d * L + L - o],
                                         AF.Copy, scale=W["cw"][d][:, j:j + 1])
                nc.vector.tensor_tensor(_segs(xc, o, L - o), _segs(xc, o, L - o),
                                        _segs(tp, 0, L - o), OP.add)
            for d in range(NDT):
                nc.scalar.activation(sb_xh[b][:, d * L:(d + 1) * L],
                                     xc[:, d * L:(d + 1) * L],
                                     AF.Silu, bias=W["cb"][d])

        # x_proj partials -> ar_in[b] -> AllReduce
        for ti, (t0, twd) in enumerate(_bslices()) if "noproj3" not in ablate else ():
            pt = ps.tile([128, XD], F32, name=f"{R}px{b}_{ti}", tag="pj",
                         bufs=8)
            for d in range(NDT):
                nc.tensor.matmul(pt[:twd, :],
                                 sb_xh[b][:, d * L + t0:d * L + t0 + twd],
                                 W["wx"][d], start=(d == 0), stop=(d == NDT - 1))
            ev = mem.tile([128, XD], BF16, name=f"{R}xde{b}_{ti}", tag="xde",
                          bufs=3)
            nc.scalar.copy(ev[:twd, :], pt[:twd, :])
            nc.sync.dma_start(out=ar_in[b][t0:t0 + twd, :], in_=ev[:twd, :])
        if cc_mode == "copy":
            nc.sync.dma_start(out=ar_out[b][:, :], in_=ar_in[b][:, :])
        elif cc_mode == "skip":
            nc.sync.dma_start(out=ar_out[b][0:1, :], in_=ar_in[b][0:1, :])
        else:
            nc.gpsimd.collective_compute(
                "AllReduce", OP.add, replica_groups=[list(range(N_CORES))],
                ins=[ar_in[b].opt()], outs=[ar_out[b].opt()])

    def trans_dt(b):
        sb_xdT = mem.tile([96, L], BF16, name=f"{R}xdT{b}", tag="xdT", bufs=2)
        if "notr" in ablate:
            nc.vector.memset(sb_xdT, 0.0)
        for ti, (t0, twd) in enumerate(_bslices()) if "notr" not in ablate else ():
            ld = mem.tile([128, XD], BF16, name=f"{R}xl{b}_{ti}", tag="xde",
                          bufs=3)
            nc.sync.dma_start(out=ld[:twd, :], in_=ar_out[b][t0:t0 + twd, :])
            pt = ps.tile([128, 128], BF16, name=f"{R}ptr{b}_{ti}", tag="pj",
                         bufs=8)
            nc.tensor.transpose(pt[:XD, :twd], ld[:twd, :XD],
                                W["ident"][:twd, :twd])
            nc.scalar.copy(sb_xdT[:, t0:t0 + twd], pt[:XD, :twd])
        if "notr" not in ablate:
            nc.sync.dma_start(out=bc_rows[b][:, :], in_=sb_xdT[DTR:XD, :])

        for d in range(NDT) if "nodt" not in ablate else ():
            for ci, (co, cw) in enumerate(BCH):
                pt = ps.tile([128, 512], F32, name=f"{R}pd{b}_{d}_{ci}",
                             tag="pj", bufs=8)
                nc.tensor.matmul(pt[:, :cw], W["wdt"][:, d * 128:(d + 1) * 128],
                                 sb_xdT[:DTR, co:co + cw], start=True, stop=True)
                e1 = mem.tile([128, 512], F32, name=f"{R}e{b}_{d}_{ci}",
                              tag="sp", bufs=3)
                nc.scalar.activation(e1[:, :cw], pt[:, :cw], AF.Exp,
                                     bias=W["bdt"][d])
                nc.scalar.activation(sb_dt[b][:, d * L + co:d * L + co + cw],
                                     e1[:, :cw], AF.Ln, bias=1.0)
        if "nodt" not in ablate:
            nc.vector.tensor_tensor(sb_dtx[b], sb_dt[b], sb_xh[b], OP.mult)
            # poison dt's segment-start columns AFTER dtx: every da =
            # exp(a_n * 1e4) = 0 there, resetting the wide scan per segment
            # without a per-n zero op (a_n <= -1 for all states)
            zv = sb_dt[b][:, 0:]
            zc = bass.AP(tensor=zv.tensor, offset=zv.offset,
                         ap=[zv.ap[0], [L, NDT]])
            nc.scalar.activation(zc, zc, AF.Copy, scale=0.0, bias=1e30)

    def block(b):
        y = mem.tile([128, WL], BF16, name=f"{R}y{b}", tag="xiy", bufs=2)
        if "noelem" in ablate or "nossm" in ablate:
            nc.vector.memset(y, 0.0)
        for n in range(NST) if "nossm" not in ablate else ():
            if "nobc" in ablate:
                bb = cb = None
                bb_t = cb_t = sb_dtx[b]
            else:
                bb = mem.tile([128, L], BF16, name=f"{R}bb{b}_{n}", tag="bb",
                              bufs=3)
                cb = mem.tile([128, L], BF16, name=f"{R}cb{b}_{n}", tag="cbx",
                              bufs=3)
                for eng, dst, row in ((nc.gpsimd, bb, n),
                                      (nc.scalar, cb, NST + n)):
                    src = bc_rows[b][row:row + 1, :]
                    eng.dma_start(out=dst, in_=bass.AP(
                        tensor=src.tensor, offset=src.offset,
                        ap=[[0, 128]] + src.ap[1:]))
            if "noelem" in ablate:
                u = sb_dtx[b]
            else:
                u = mem.tile([128, WL], BF16, name=f"{R}u{b}_{n}", tag="u",
                             bufs=3)
                in1 = _rep(bb, NDT) if bb is not None else bb_t[:, :]
                eng = nc.gpsimd if "upool" in ablate else nc.vector
                eng.tensor_tensor(u, sb_dtx[b][:, :], in1, OP.mult)
            da = mem.tile([128, WL], BF16, name=f"{R}da{b}_{n}", tag="da",
                          bufs=3)
            if a_scales is not None:
                nc.scalar.activation(da, sb_dt[b], AF.Exp,
                                     scale=float(a_scales[n]))
            else:
                for d in range(NDT):
                    nc.scalar.activation(da[:, d * L:(d + 1) * L],
                                         sb_dt[b][:, d * L:(d + 1) * L],
                                         AF.Exp, scale=W["acol"][d][:, n:n + 1])
            if "noscan" in ablate:
                h = u
            else:
                h = mem.tile([128, WL], BF16, name=f"{R}h{b}_{n}", tag="h",
                             bufs=2)
                nc.vector.tensor_tensor_scan(h, da, u, 0.0, OP.mult, OP.add)
            if "noelem" in ablate:
                continue
            p = mem.tile([128, WL], BF16, name=f"{R}p{b}_{n}", tag="p", bufs=2)
            in1 = _rep(cb, NDT) if cb is not None else cb_t[:, :]
            nc.vector.tensor_tensor(p, h[:, :], in1, OP.mult)
            if n == 0:
                nc.vector.tensor_copy(y, p)
            else:
                nc.vector.tensor_tensor(y, y, p, OP.add)
        return y

    def gate_out(b, y):
        s = b * L
        sk = mem.tile([128, WL], BF16, name=f"{R}sk{b}", tag="u", bufs=3)
        for d in range(NDT):
            nc.vector.tensor_scalar_mul(sk[:, d * L:(d + 1) * L],
                                        sb_xh[b][:, d * L:(d + 1) * L],
                                        W["dsk"][d][:, 0:1])
        nc.vector.tensor_tensor(sk, y, sk, OP.add)
        nc.vector.tensor_tensor(sb_z[b], sk, sb_z[b], OP.mult)

        for ti, (t0, twd) in enumerate(_bslices()) if "noout" not in ablate else ():
            for f in range(2):
                pt = ps.tile([128, 512], F32, name=f"{R}po{b}_{ti}_{f}",
                             tag="pj", bufs=8)
                for d in range(NDT):
                    nc.tensor.matmul(
                        pt[:twd, :], sb_z[b][:, d * L + t0:d * L + t0 + twd],
                        W["wo"][d][:, f * 512:(f + 1) * 512],
                        start=(d == 0), stop=(d == NDT - 1))
                ev = mem.tile([128, 512], F32, name=f"{R}oe{b}_{ti}_{f}",
                              tag="sp", bufs=3)
                nc.scalar.copy(ev[:twd, :], pt[:twd, :])
                nc.sync.dma_start(
                    out=out_p[s + t0:s + t0 + twd, f * 512:(f + 1) * 512],
                    in_=ev[:twd, :])

    front(0)
    front(1)
    trans_dt(0)
    y0 = block(0)
    trans_dt(1)          # PE/Act overlap with block(0); before out_proj(0)
    gate_out(0, y0)
    y1 = block(1)
    gate_out(1, y1)


_CACHE = {}


def _get_program(a_scales_key, **kw):
    key = (a_scales_key, tuple(sorted(kw.items())))
    if key not in _CACHE:
        _CACHE[key] = build_program(
            list(a_scales_key) if a_scales_key is not None else None, **kw)
    return _CACHE[key]


def make_inputs(x, layer_idx, emb_table, W_in, conv_w, conv_b, W_x, W_dt, b_dt,
                A_log, D_skip, W_out):
    x = np.asarray(x, np.float32)
    emb = np.asarray(emb_table, np.float32)[int(layer_idx)]
    xs = np.concatenate([np.broadcast_to(emb, (B, 1, DM)), x], axis=1)
    xsT = np.ascontiguousarray(xs.reshape(TT, DM).T).astype(ml_dtypes.bfloat16)

    A = -np.exp(np.asarray(A_log, np.float64)).astype(np.float32)
    same = bool(np.all(A == A[0:1, :]))
    a_key = tuple(float(v) for v in A[0]) if same else None

    W_in = np.asarray(W_in, np.float32)
    identb = np.eye(128, dtype=ml_dtypes.bfloat16)
    ins = []
    for c in range(N_CORES):
        sl = slice(c * DLOC, (c + 1) * DLOC)
        w_in_cat = np.concatenate(
            [W_in[:, c * DLOC:(c + 1) * DLOC],
             W_in[:, DI + c * DLOC:DI + (c + 1) * DLOC]], axis=1)
        ins.append({
            "xsT": xsT,
            "w_in": np.ascontiguousarray(w_in_cat).astype(ml_dtypes.bfloat16),
            "conv_w": np.ascontiguousarray(np.asarray(conv_w, np.float32)[sl]),
            "conv_b": np.ascontiguousarray(
                np.asarray(conv_b, np.float32)[sl][:, None]),
            "w_x": np.ascontiguousarray(np.asarray(W_x, np.float32)[sl]).astype(ml_dtypes.bfloat16),
            "w_dt": np.ascontiguousarray(np.asarray(W_dt, np.float32)[:, sl]).astype(ml_dtypes.bfloat16),
            "b_dt": np.ascontiguousarray(
                np.asarray(b_dt, np.float32)[sl][:, None]),
            "a_cols": np.ascontiguousarray(A[sl]),
            "d_skip": np.ascontiguousarray(
                np.asarray(D_skip, np.float32)[sl][:, None]),
            "w_out": np.ascontiguousarray(np.asarray(W_out, np.float32)[sl]).astype(ml_dtypes.bfloat16),
            "identb": identb,
        })
    return ins, a_key


def kernel(**inputs) -> np.ndarray:
    ins, a_key = make_inputs(**inputs)
    nc = _get_program(a_key)
    res = run_bass_kernel_spmd(nc, ins, core_ids=list(range(N_CORES)))
    out = np.zeros((TT, DM), np.float64)
    for c in range(N_CORES):
        out += res.results[c]["out_p"]
    return out.astype(np.float32).reshape(B, L, DM)
